# revision 6
# baseline (speedup 1.0000x reference)
"""AFGNN layer (6-hop sparse message passing + softmax mix + dense proj) on
8 TRN2 NeuronCores.

Strategy:
  - Row-shard the 100352 (padded) nodes: 12544 rows per core (98 tiles of 128).
  - Fold the softmax mix into per-edge values; self-loops become edges with
    val = mix[0]. Each core gets the ~460k edges targeting its rows, sorted by
    (row-tile, col-chunk) and padded to 128-edge blocks with (col=0, sval=0)
    dummies so all 8 cores share one compiled graph.
  - Input features are replicated to every core's HBM as bf16 [100352, 128].
  - Per 128-edge block: SWDGE dma_gather fetches the 128 source rows (bf16,
    256B each); DVE builds P[e,r] = sval_e * (slot_e == r) in one fused
    tensor_scalar; TensorE accumulates repT_tile += G_b^T P_b in PSUM.
  - Epilogue per row-tile: repT -> bf16, out = rep @ W + bias and
    rep = transpose(repT) via TensorE, DMA straight from PSUM to DRAM.
"""

import numpy as np
import ml_dtypes

N = 100000
NPAD = 100352          # 784 * 128
D = 128
NCORES = 8
RPC = NPAD // NCORES   # 12544 rows per core
NT = RPC // 128        # 98 row tiles per core
CHUNK = 32768          # int16-index col chunks
NCH = 4                # ceil(NPAD / CHUNK)

_cache = {}


def _prep(input, adj_rows, adj_cols, adj_vals, weight, linear_weight, bias):
    """Host-side sharding: per-core padded edge streams + replicated consts."""
    f32 = np.float32
    bf16 = ml_dtypes.bfloat16

    lw = np.asarray(linear_weight, np.float64)
    e = np.exp(lw - lw.max())
    mix = (e / e.sum()).astype(f32)          # [7]

    rows = np.concatenate([np.asarray(adj_rows).reshape(-1),
                           np.arange(N, dtype=np.int64)])
    cols = np.concatenate([np.asarray(adj_cols).reshape(-1),
                           np.arange(N, dtype=np.int64)])
    sval = np.concatenate([
        (np.asarray(adj_vals, f32) * mix[1:, None]).reshape(-1),
        np.full(N, mix[0], f32)])

    core = (rows // RPC).astype(np.int64)

    # per-core sorted edge arrays and (tile, chunk) counts
    per_core = []
    counts = np.zeros((NCORES, NT * NCH), np.int64)
    for k in range(NCORES):
        m = core == k
        r = (rows[m] - k * RPC).astype(np.int32)
        c = cols[m].astype(np.int32)
        v = sval[m]
        t = r >> 7
        ch = c >> 15
        seg = t * NCH + ch
        order = np.argsort(seg, kind="stable")
        seg = seg[order]
        counts[k] = np.bincount(seg, minlength=NT * NCH)
        per_core.append((r[order], c[order], v[order], seg))

    B = np.ceil(counts.max(axis=0) / 128).astype(np.int64)  # blocks per segment
    B = np.maximum(B, 1)
    seg_slots = B * 128
    seg_start = np.concatenate([[0], np.cumsum(seg_slots)])
    epad = int(seg_start[-1])
    nblk = epad // 128

    xin = np.zeros((NPAD, D), bf16)
    xin[:N] = np.asarray(input, f32).astype(bf16)

    wmat = np.asarray(weight, f32).astype(bf16)            # [D, D] rhs
    bias_b = np.asarray(bias, f32).astype(bf16)[None, :]   # [1, D]
    ones_b = np.ones((1, D), bf16)
    iota = np.broadcast_to(np.arange(D, dtype=f32).astype(bf16), (D, D)).copy()
    ident = np.eye(D, dtype=bf16)

    in_maps = []
    for k in range(NCORES):
        r, c, v, seg = per_core[k]
        ncnt = counts[k]
        # destination slot of each real edge in the padded stream
        within = np.arange(len(r)) - np.repeat(np.concatenate([[0], np.cumsum(ncnt)[:-1]]), ncnt)
        dest = seg_start[seg] + within

        cpad = np.zeros(epad, np.int32)            # pad col -> local 0 (valid)
        spad = np.full(epad, -1.0, f32)            # pad slot -> never matches
        vpad = np.zeros(epad, f32)
        cpad[dest] = c & 32767
        spad[dest] = (r & 127).astype(f32)
        vpad[dest] = v

        # wrap indices per segment: idx i -> [i % 16, i // 16]
        gidx16 = np.empty((16, epad // 16), np.int16)
        for s in range(NT * NCH):
            a, b = seg_start[s], seg_start[s + 1]
            gidx16[:, a // 16:b // 16] = cpad[a:b].reshape(-1, 16).T
        gidx = np.broadcast_to(gidx16, (8, 16, epad // 16)).reshape(128, epad // 16).copy()

        in_maps.append({
            "xin": xin,
            "gidx": gidx,
            "slot": spad.reshape(nblk, 128).T.copy(),   # [128, nblk] f32
            "sval": vpad.reshape(nblk, 128).T.copy(),
            "wmat": wmat,
            "biasb": bias_b,
            "onesb": ones_b,
            "iota": iota,
            "ident": ident,
        })
    return in_maps, B.reshape(NT, NCH), epad


def _build(B, nt_limit=NT):
    import concourse.bass as bass
    import concourse.bacc as bacc
    import concourse.mybir as mybir
    import concourse.tile as tile

    dt = mybir.dt
    nblk = int(B.sum())
    epad = nblk * 128

    nc = bacc.Bacc(None, num_swdge_queues=4)
    xin_d = nc.declare_dram_parameter("xin", [NPAD, D], dt.bfloat16, isOutput=False)
    gidx_d = nc.declare_dram_parameter("gidx", [128, epad // 16], dt.int16, isOutput=False)
    slot_d = nc.declare_dram_parameter("slot", [128, nblk], dt.float32, isOutput=False)
    sval_d = nc.declare_dram_parameter("sval", [128, nblk], dt.float32, isOutput=False)
    wmat_d = nc.declare_dram_parameter("wmat", [D, D], dt.bfloat16, isOutput=False)
    bias_d = nc.declare_dram_parameter("biasb", [1, D], dt.bfloat16, isOutput=False)
    ones_d = nc.declare_dram_parameter("onesb", [1, D], dt.bfloat16, isOutput=False)
    iota_d = nc.declare_dram_parameter("iota", [D, D], dt.bfloat16, isOutput=False)
    ident_d = nc.declare_dram_parameter("ident", [D, D], dt.bfloat16, isOutput=False)
    out_d = nc.declare_dram_parameter("out", [RPC, D], dt.float32, isOutput=True)
    rep_d = nc.declare_dram_parameter("rep", [RPC, D], dt.float32, isOutput=True)

    with tile.TileContext(nc) as tc:
        with (
            tc.tile_pool(name="const", bufs=1) as const,
            tc.tile_pool(name="adj", bufs=1) as adj,
            tc.tile_pool(name="gbuf", bufs=4) as gbuf,
            tc.tile_pool(name="pbuf", bufs=6) as pbuf,
            tc.tile_pool(name="rbuf", bufs=3) as rbuf,
            tc.tile_pool(name="ps_acc", bufs=3, space="PSUM") as ps_acc,
            tc.tile_pool(name="ps_out", bufs=2, space="PSUM") as ps_out,
            tc.tile_pool(name="ps_rep", bufs=2, space="PSUM") as ps_rep,
        ):
            wmat = const.tile([D, D], dt.bfloat16)
            biasb = const.tile([1, D], dt.bfloat16)
            onesb = const.tile([1, D], dt.bfloat16)
            iota = const.tile([D, D], dt.bfloat16)
            ident = const.tile([D, D], dt.bfloat16)
            gidx = adj.tile([128, epad // 16], dt.int16)
            slot = adj.tile([128, nblk], dt.float32)
            sval = adj.tile([128, nblk], dt.float32)

            nc.sync.dma_start(wmat[:], wmat_d[:])
            nc.sync.dma_start(biasb[:], bias_d[:])
            nc.sync.dma_start(onesb[:], ones_d[:])
            nc.sync.dma_start(iota[:], iota_d[:])
            nc.sync.dma_start(ident[:], ident_d[:])
            nc.sync.dma_start(gidx[:], gidx_d[:])
            nc.sync.dma_start(slot[:], slot_d[:])
            nc.sync.dma_start(sval[:], sval_d[:])

            g = 0       # global block counter
            off = 0     # index-stream offset (units of 16 idxs)
            qn = 0      # SWDGE queue rotation
            for t in range(nt_limit):
                acc = ps_acc.tile([D, D], dt.float32)
                bt = int(B[t].sum())
                bi = 0
                for c in range(NCH):
                    bc = int(B[t, c])
                    if bc == 0:
                        continue
                    gt = gbuf.tile([128, bc * 128], dt.bfloat16, tag="gt")
                    gt3 = gt[:].rearrange("p (b e) -> p b e", e=128)
                    for b0 in range(0, bc, 8):
                        bn = min(8, bc - b0)
                        nc.gpsimd.dma_gather(
                            out_ap=gt3[:, b0:b0 + bn, :],
                            in_ap=xin_d[c * CHUNK:min((c + 1) * CHUNK, NPAD), :],
                            idxs_ap=gidx[:, off + b0 * 8:off + (b0 + bn) * 8],
                            num_idxs=bn * 128,
                            num_idxs_reg=bn * 128,
                            elem_size=D,
                            queue_num=qn % 4,
                        )
                        qn += 1
                    off += bc * 8
                    for b in range(bc):
                        p = pbuf.tile([128, D], dt.bfloat16, tag="p")
                        nc.vector.tensor_scalar(
                            p[:], iota[:],
                            slot[:, g:g + 1], sval[:, g:g + 1],
                            mybir.AluOpType.is_equal, mybir.AluOpType.mult,
                        )
                        nc.tensor.matmul(
                            acc[:], gt3[:, b, :], p[:],
                            start=(bi == 0), stop=(bi == bt - 1),
                        )
                        g += 1
                        bi += 1
                # epilogue for row tile t
                rbf = rbuf.tile([D, D], dt.bfloat16, tag="rbf")
                nc.vector.tensor_copy(rbf[:], acc[:])
                outp = ps_out.tile([D, D], dt.float32)
                nc.tensor.matmul(outp[:], rbf[:], wmat[:], start=True, stop=False)
                nc.tensor.matmul(outp[:], onesb[:], biasb[:], start=False, stop=True)
                repp = ps_rep.tile([D, D], dt.float32)
                nc.tensor.matmul(repp[:], rbf[:], ident[:], start=True, stop=True)
                outs = rbuf.tile([D, D], dt.float32, tag="outs")
                reps = rbuf.tile([D, D], dt.float32, tag="reps")
                nc.scalar.copy(outs[:], outp[:])
                nc.scalar.copy(reps[:], repp[:])
                nc.sync.dma_start(out_d[t * 128:(t + 1) * 128, :], outs[:])
                nc.sync.dma_start(rep_d[t * 128:(t + 1) * 128, :], reps[:])

    nc.compile()
    return nc


def kernel(**inputs):
    from concourse.bass_utils import run_bass_kernel_spmd

    in_maps, B, epad = _prep(**inputs)
    key = tuple(B.reshape(-1))
    if key not in _cache:
        _cache.clear()
        _cache[key] = _build(B)
    nc = _cache[key]

    res = run_bass_kernel_spmd(nc, in_maps, list(range(NCORES)))
    out = np.concatenate([np.asarray(res.results[k]["out"]) for k in range(NCORES)])
    rep = np.concatenate([np.asarray(res.results[k]["rep"]) for k in range(NCORES)])
    return out[:N].astype(np.float32), rep[:N].astype(np.float32)


# revision 12
# speedup vs baseline: 1.8396x; 1.8396x over previous
"""AFGNN layer (6-hop sparse message passing + softmax mix + dense proj) on
8 TRN2 NeuronCores.

Strategy:
  - Row-shard the 100352 (padded) nodes: 12544 rows per core (98 tiles of 128).
  - Fold the softmax mix into per-edge values. The self-loop (mix0 * input)
    term is NOT materialized as edges; it is folded into each row-tile's PSUM
    accumulation as one extra matmul against a resident local-input copy.
  - Input features are replicated to every core's HBM as bf16 [100352, 128].
  - Edges sorted by (col-chunk, row-tile): the gather stream is chunk-major so
    SWDGE dma_gather calls run at the full 1024-descriptor ring size across
    row-tile boundaries (per-call overhead amortized). Four passes (one per
    32768-node col chunk, int16 index limit); each pass accumulates per-tile
    partials in PSUM via the one-hot matmul and adds them into an SBUF f32
    repT accumulator.
  - Per 128-edge block: DVE builds P[e,r] = sval_e * (slot_e == r) via two
    broadcast tensor_tensor ops batched over 32-block windows; TensorE does
    repT_partial += G_b^T @ P_b.
  - Epilogue per row-tile: repT -> bf16, out = rep @ W + bias and
    rep = transpose(repT) via TensorE, staged to SBUF, DMA to DRAM.
"""

import numpy as np
import ml_dtypes

N = 100000
NPAD = 100352          # 784 * 128
D = 128
NCORES = 8
RPC = NPAD // NCORES   # 12544 rows per core
NT = RPC // 128        # 98 row tiles per core
CHUNK = 32768          # int16-index col chunks
NCH = 4                # ceil(NPAD / CHUNK)
CALL = 1024            # max descriptors per SWDGE gather call (ring depth)
PWIN = 32              # P-build window, in 128-edge blocks

_cache = {}


def _prep(input, adj_rows, adj_cols, adj_vals, weight, linear_weight, bias):
    """Host-side sharding: per-core padded edge streams + replicated consts."""
    f32 = np.float32
    bf16 = ml_dtypes.bfloat16

    lw = np.asarray(linear_weight, np.float64)
    e = np.exp(lw - lw.max())
    mix = (e / e.sum()).astype(f32)          # [7]

    rows = np.asarray(adj_rows).reshape(-1)
    cols = np.asarray(adj_cols).reshape(-1)
    sval = (np.asarray(adj_vals, f32) * mix[1:, None]).reshape(-1)

    core = rows // RPC

    # per-core edge arrays sorted by (chunk, tile) and (chunk, tile) counts
    per_core = []
    counts = np.zeros((NCORES, NCH * NT), np.int64)
    for k in range(NCORES):
        m = core == k
        r = (rows[m] - k * RPC).astype(np.int32)
        c = cols[m].astype(np.int32)
        v = sval[m]
        t = r >> 7
        ch = c >> 15
        seg = ch * NT + t                    # chunk-major segments
        order = np.argsort(seg, kind="stable")
        seg = seg[order]
        counts[k] = np.bincount(seg, minlength=NCH * NT)
        per_core.append((r[order], c[order], v[order], seg))

    B = np.ceil(counts.max(axis=0) / 128).astype(np.int64)  # blocks per segment
    B = np.maximum(B, 1)
    seg_slots = B * 128
    seg_start = np.concatenate([[0], np.cumsum(seg_slots)])
    epad = int(seg_start[-1])
    nblk = epad // 128

    xin = np.zeros((NPAD, D), bf16)
    xin[:N] = np.asarray(input, f32).astype(bf16)

    wmat = np.asarray(weight, f32).astype(bf16)            # [D, D] rhs
    bias_b = np.asarray(bias, f32).astype(bf16)[None, :]   # [1, D]
    ones_b = np.ones((1, D), bf16)
    iota = np.broadcast_to(np.arange(D, dtype=f32).astype(bf16), (D, D)).copy()
    ident = np.eye(D, dtype=bf16)
    mix0i = (np.eye(D, dtype=np.float64) * mix[0]).astype(bf16)

    in_maps = []
    for k in range(NCORES):
        r, c, v, seg = per_core[k]
        ncnt = counts[k]
        within = np.arange(len(r)) - np.repeat(np.concatenate([[0], np.cumsum(ncnt)[:-1]]), ncnt)
        dest = seg_start[seg] + within

        cpad = np.zeros(epad, np.int32)            # pad col -> local 0 (valid)
        spad = np.full(epad, -1.0, f32)            # pad slot -> never matches
        vpad = np.zeros(epad, f32)
        cpad[dest] = c & 32767
        spad[dest] = (r & 127).astype(f32)
        vpad[dest] = v

        # wrap indices per 16: call slices stay consistent because every call
        # starts at a multiple of 128 indices
        gidx16 = cpad.reshape(-1, 16).T.astype(np.int16)   # [16, epad//16]
        gidx = np.broadcast_to(gidx16, (8, 16, epad // 16)).reshape(128, epad // 16).copy()

        in_maps.append({
            "xin": xin,
            "gidx": gidx,
            "slot": spad.reshape(nblk, 128).T.copy(),   # [128, nblk] f32
            "sval": vpad.reshape(nblk, 128).T.copy(),
            "xloc": np.ascontiguousarray(xin[k * RPC:(k + 1) * RPC]),
            "wmat": wmat,
            "biasb": bias_b,
            "onesb": ones_b,
            "iota": iota,
            "ident": ident,
            "mix0i": mix0i,
        })
    return in_maps, B.reshape(NCH, NT), epad


def _build(B, nt_limit=NT):
    import concourse.bass as bass
    import concourse.bacc as bacc
    import concourse.mybir as mybir
    import concourse.tile as tile

    dt = mybir.dt
    nblk = int(B.sum())
    epad = nblk * 128

    nc = bacc.Bacc(None, num_swdge_queues=4)
    xin_d = nc.declare_dram_parameter("xin", [NPAD, D], dt.bfloat16, isOutput=False)
    gidx_d = nc.declare_dram_parameter("gidx", [128, epad // 16], dt.int16, isOutput=False)
    slot_d = nc.declare_dram_parameter("slot", [128, nblk], dt.float32, isOutput=False)
    sval_d = nc.declare_dram_parameter("sval", [128, nblk], dt.float32, isOutput=False)
    xloc_d = nc.declare_dram_parameter("xloc", [RPC, D], dt.bfloat16, isOutput=False)
    wmat_d = nc.declare_dram_parameter("wmat", [D, D], dt.bfloat16, isOutput=False)
    bias_d = nc.declare_dram_parameter("biasb", [1, D], dt.bfloat16, isOutput=False)
    ones_d = nc.declare_dram_parameter("onesb", [1, D], dt.bfloat16, isOutput=False)
    iota_d = nc.declare_dram_parameter("iota", [D, D], dt.bfloat16, isOutput=False)
    ident_d = nc.declare_dram_parameter("ident", [D, D], dt.bfloat16, isOutput=False)
    mix0i_d = nc.declare_dram_parameter("mix0i", [D, D], dt.bfloat16, isOutput=False)
    out_d = nc.declare_dram_parameter("out", [RPC, D], dt.float32, isOutput=True)
    rep_d = nc.declare_dram_parameter("rep", [RPC, D], dt.float32, isOutput=True)

    # static plan: gather calls merge blocks within one chunk, up to CALL idxs
    calls = []              # (chunk, first_block, n_blocks)
    b0 = 0
    for c in range(NCH):
        cb = int(B[c].sum())
        q = 0
        while q < cb:
            nb = min(CALL // 128, cb - q)
            calls.append((c, b0 + q, nb))
            q += nb
        b0 += cb

    with tile.TileContext(nc) as tc:
        with (
            tc.tile_pool(name="const", bufs=1) as const,
            tc.tile_pool(name="adj", bufs=1) as adj,
            tc.tile_pool(name="gbuf", bufs=8) as gbuf,
            tc.tile_pool(name="pbuf", bufs=4) as pbuf,
            tc.tile_pool(name="rbuf", bufs=4) as rbuf,
            tc.tile_pool(name="racc", bufs=1) as racc,
            tc.tile_pool(name="ps_acc", bufs=4, space="PSUM") as ps_acc,
            tc.tile_pool(name="ps_out", bufs=2, space="PSUM") as ps_out,
            tc.tile_pool(name="ps_rep", bufs=2, space="PSUM") as ps_rep,
        ):
            wmat = const.tile([D, D], dt.bfloat16)
            biasb = const.tile([1, D], dt.bfloat16)
            onesb = const.tile([1, D], dt.bfloat16)
            iota = const.tile([D, D], dt.bfloat16)
            ident = const.tile([D, D], dt.bfloat16)
            mix0i = const.tile([D, D], dt.bfloat16)
            xloc = const.tile([128, NT, D], dt.bfloat16)
            gidx = adj.tile([128, epad // 16], dt.int16)
            slot = adj.tile([128, nblk], dt.float32)
            sval = adj.tile([128, nblk], dt.float32)
            repT = racc.tile([128, NT, D], dt.bfloat16)

            nc.sync.dma_start(wmat[:], wmat_d[:])
            nc.sync.dma_start(biasb[:], bias_d[:])
            nc.sync.dma_start(onesb[:], ones_d[:])
            nc.sync.dma_start(iota[:], iota_d[:])
            nc.sync.dma_start(ident[:], ident_d[:])
            nc.sync.dma_start(mix0i[:], mix0i_d[:])
            nc.sync.dma_start(xloc[:], xloc_d[:].rearrange("(t p) d -> p t d", p=128))
            nc.sync.dma_start(gidx[:], gidx_d[:])
            nc.sync.dma_start(slot[:], slot_d[:])
            nc.sync.dma_start(sval[:], sval_d[:])

            # P one-hot build in PWIN-block windows
            pts = {}            # global block -> (pt3 handle, offset)
            for w0 in range(0, nblk, PWIN):
                wn = min(PWIN, nblk - w0)
                pt = pbuf.tile([128, wn * 128], dt.bfloat16, tag="p")
                pt3 = pt[:].rearrange("p (b r) -> p b r", r=128)
                iap = iota[:]
                iota_bc = bass.AP(iap.tensor, iap.offset, [iap.ap[0], [0, wn], iap.ap[1]])
                sl = slot[:, w0:w0 + wn]
                slot_bc = bass.AP(sl.tensor, sl.offset, [sl.ap[0], sl.ap[1], [0, 128]])
                sv = sval[:, w0:w0 + wn]
                sval_bc = bass.AP(sv.tensor, sv.offset, [sv.ap[0], sv.ap[1], [0, 128]])
                nc.vector.tensor_tensor(pt3, slot_bc, iota_bc, mybir.AluOpType.is_equal)
                nc.vector.tensor_tensor(pt3, pt3, sval_bc, mybir.AluOpType.mult)
                for j in range(wn):
                    pts[w0 + j] = (pt3, j)

            # gather calls (chunk-major, up to 1024 descriptors each)
            gts = {}            # global block -> (gt3 handle, offset)
            for qn, (c, blk0, nb) in enumerate(calls):
                gt = gbuf.tile([128, nb * 128], dt.bfloat16, tag="gt")
                gt3 = gt[:].rearrange("p (b e) -> p b e", e=128)
                nc.gpsimd.dma_gather(
                    out_ap=gt3,
                    in_ap=xin_d[c * CHUNK:min((c + 1) * CHUNK, NPAD), :],
                    idxs_ap=gidx[:, blk0 * 8:(blk0 + nb) * 8],
                    num_idxs=nb * 128,
                    num_idxs_reg=nb * 128,
                    elem_size=D,
                    queue_num=qn % 4,
                )
                for j in range(nb):
                    gts[blk0 + j] = (gt3, j)

            # per-(chunk, tile) PSUM accumulation folded into SBUF f32 repT
            g = 0
            for c in range(NCH):
                for t in range(NT):
                    bc = int(B[c, t])
                    if t >= nt_limit:
                        g += bc
                        continue
                    acc = ps_acc.tile([D, D], dt.float32)
                    extra = (c == NCH - 1)   # fold mix0 * x_local on last pass
                    for j in range(bc):
                        gt3, gj = gts[g + j]
                        pt3, pj = pts[g + j]
                        nc.tensor.matmul(
                            acc[:], gt3[:, gj, :], pt3[:, pj, :],
                            start=(j == 0), stop=(j == bc - 1 and not extra),
                        )
                    if extra:
                        nc.tensor.matmul(
                            acc[:], xloc[:, t, :], mix0i[:],
                            start=False, stop=True,
                        )
                    g += bc
                    if c == 0:
                        nc.vector.tensor_copy(repT[:, t, :], acc[:])
                    else:
                        nc.vector.tensor_add(repT[:, t, :], repT[:, t, :], acc[:])

            # epilogue per tile
            for t in range(nt_limit):
                rbf = rbuf.tile([D, D], dt.bfloat16, tag="rbf")
                nc.scalar.copy(rbf[:], repT[:, t, :])
                outp = ps_out.tile([D, D], dt.float32)
                nc.tensor.matmul(outp[:], rbf[:], wmat[:], start=True, stop=False)
                nc.tensor.matmul(outp[:], onesb[:], biasb[:], start=False, stop=True)
                repp = ps_rep.tile([D, D], dt.float32)
                nc.tensor.matmul(repp[:], rbf[:], ident[:], start=True, stop=True)
                outs = rbuf.tile([D, D], dt.float32, tag="outs")
                reps = rbuf.tile([D, D], dt.float32, tag="reps")
                nc.scalar.copy(outs[:], outp[:])
                nc.scalar.copy(reps[:], repp[:])
                nc.sync.dma_start(out_d[t * 128:(t + 1) * 128, :], outs[:])
                nc.sync.dma_start(rep_d[t * 128:(t + 1) * 128, :], reps[:])

    nc.compile()
    return nc


def kernel(**inputs):
    from concourse.bass_utils import run_bass_kernel_spmd

    in_maps, B, epad = _prep(**inputs)
    key = tuple(B.reshape(-1))
    if key not in _cache:
        _cache.clear()
        _cache[key] = _build(B)
    nc = _cache[key]

    res = run_bass_kernel_spmd(nc, in_maps, list(range(NCORES)))
    out = np.concatenate([np.asarray(res.results[k]["out"]) for k in range(NCORES)])
    rep = np.concatenate([np.asarray(res.results[k]["rep"]) for k in range(NCORES)])
    return out[:N].astype(np.float32), rep[:N].astype(np.float32)
